# revision 17
# baseline (speedup 1.0000x reference)
"""Trainium2 Bass kernel for nn_CrossAttentionLayer (8-core SPMD).

  q  = x1 @ W1.T + b1            [N1, D2]
  attn = softmax(q @ x2.T, dim=1)
  fused_x1 = attn @ x2           [N1, D2]
  fused_x2 = attn.T @ x1         [N2, D1]   (ReduceScatter over row shards)

Sharding: x1 rows split across 8 cores (1024 rows each); x2/W1 replicated.
Per core: scores computed twice on the PE (row-major for fused_x2's operand,
transposed per key-tile for fused_x1) in float32r; exp on ACT; attention
weights kept unnormalized in bf16, with softmax row sums obtained for free
from the ACT accum_out, applied as inv-sum scaling of x1 (fused_x2) and of
the fused_x1 result after an on-PE transpose. fused_x2 partials are
ReduceScattered on-device; the host concatenates the per-core shards.
"""

import sys

if "/opt/trn_rl_repo" not in sys.path:
    sys.path.insert(0, "/opt/trn_rl_repo")

import numpy as np
import ml_dtypes

import concourse.bacc as bacc
import concourse.mybir as mybir
import concourse.tile as tile
from concourse.bass_utils import run_bass_kernel_spmd
from concourse.masks import make_identity

F32 = mybir.dt.float32
F32R = mybir.dt.float32r
BF16 = mybir.dt.bfloat16
FP16 = mybir.dt.float16
EXPF = mybir.ActivationFunctionType.Exp
IDF = mybir.ActivationFunctionType.Identity

N1, N2 = 8192, 8192
D1, D2 = 256, 128
NC = 8
R = N1 // NC          # rows per core: 1024
RT = R // 128         # row tiles per core: 8
KT = N2 // 128        # key tiles: 64
KS = N2 // 512        # 512-wide key strips: 16

_CACHED = {}


def _build(rep=1, for_sim=False, no_rs=False):
    no_rs = no_rs or for_sim
    nd = 1 if for_sim else NC
    nc = bacc.Bacc("TRN2", target_bir_lowering=False, debug=False, num_devices=nd)

    # Kernel I/O (per core). f32r tensors receive raw fp32 bytes; the PE's
    # f32r mode reads them at full rate (validated: same accuracy as a DVE
    # round-to-f32r pass).
    x1t_io = nc.dram_tensor("x1t", [D1, R], F32R, kind="ExternalInput").ap()
    x1b_io = nc.dram_tensor("x1b", [R, D1], BF16, kind="ExternalInput").ap()
    x2t_io = nc.dram_tensor("x2t", [D2, N2], FP16, kind="ExternalInput").ap()
    x2b_io = nc.dram_tensor("x2b", [N2, D2], BF16, kind="ExternalInput").ap()
    w1t_io = nc.dram_tensor("w1t", [D1, D2], F32R, kind="ExternalInput").ap()
    b1_io = nc.dram_tensor("b1", [D2, 1], F32, kind="ExternalInput").ap()
    fx1_io = nc.dram_tensor("fx1", [R, D2], F32, kind="ExternalOutput").ap()
    fx2s_io = nc.dram_tensor("fx2s", [D1 // NC, N2], F32, kind="ExternalOutput").ap()

    with tile.TileContext(nc) as tc:
        with (
            tc.tile_pool(name="pers", bufs=1) as pers,
            tc.tile_pool(name="dram", bufs=1, space="DRAM") as drp,
        ):
            # ---- persistent SBUF ----
            x2t_sb = pers.tile([128, N2], FP16)           # x2.T  (j part, keys free)
            x2b_sb = pers.tile([128, KT, D2], BF16)       # x2    (key%128 part, kt, j)
            x1b_sb = pers.tile([128, RT, D1], BF16)       # x1, scaled by inv_sum in place
            qt_sb = pers.tile([128, R], FP16)             # q.T   (j part, rows free)
            exp_sb = pers.tile([128, RT, N2], BF16)       # exp(scores) (row%128, rt, key)
            ssum = pers.tile([128, RT, 8], F32)           # per-strip exp sums
            sums = pers.tile([128, RT], F32)
            inv = pers.tile([128, RT], F32)
            ident = pers.tile([128, 128], F32)
            identb = pers.tile([128, 128], BF16)
            u1sb = pers.tile([128, R], F32)               # unnormalized fused_x1.T

            # fused_x2.T partials & ReduceScatter buffers (split in key
            # quarters so collectives overlap the tail of D)
            fx2t_d = [drp.tile([D1, N2 // 4], F32, name=f"fx2t_{h}") for h in range(4)]
            rs_out = [drp.tile([D1 // NC, N2 // 4], F32, name=f"rso_{h}") for h in range(4)]

            make_identity(nc, ident[:])
            make_identity(nc, identb[:])

            for ri in range(rep):
                _emit_iter(
                    nc, tc, pers,
                    x1t_io, x1b_io, x2t_io, x2b_io, w1t_io, b1_io,
                    fx1_io, fx2s_io,
                    x2t_sb, x2b_sb, x1b_sb, qt_sb, exp_sb,
                    ssum, sums, inv, ident, identb, u1sb, fx2t_d, rs_out,
                    no_rs,
                )

    nc.compile()
    return nc


def _emit_iter(
    nc, tc, pers,
    x1t_io, x1b_io, x2t_io, x2b_io, w1t_io, b1_io, fx1_io, fx2s_io,
    x2t_sb, x2b_sb, x1b_sb, qt_sb, exp_sb,
    ssum, sums, inv, ident, identb, u1sb, fx2t_d, rs_out,
    no_rs=False,
):
    with (
        tc.tile_pool(name="stage", bufs=1) as stage,
        tc.tile_pool(name="psQ", bufs=1, space="PSUM") as psQ,
    ):
        x1t_sb = stage.tile([128, 2, R], F32R)
        w1t_sb = stage.tile([128, 2, D2], F32R)
        b1_sb = stage.tile([128, 1], F32)
        # q-critical loads first, then x2t in chunks so the first score
        # matmuls start early; bulk loads for later phases go via gpsimd.
        nc.sync.dma_start(w1t_sb[:], w1t_io.rearrange("(dh p) j -> p dh j", p=128))
        nc.sync.dma_start(b1_sb[:], b1_io)
        nc.sync.dma_start(x1t_sb[:], x1t_io.rearrange("(dh p) m -> p dh m", p=128))
        for c in range(8):
            nc.sync.dma_start(
                x2t_sb[:, c * 1024 : (c + 1) * 1024],
                x2t_io[:, c * 1024 : (c + 1) * 1024],
            )
        nc.gpsimd.dma_start(x1b_sb[:], x1b_io.rearrange("(rt p) d -> p rt d", p=128))
        nc.gpsimd.dma_start(x2b_sb[:], x2b_io.rearrange("(kt p) j -> p kt j", p=128))

        # ---- q.T = W1T.T @ x1T + b1 : [128 j, 1024 m] ----
        q_ps = psQ.tile([128, R], F32)
        for s in range(2):
            sl = slice(s * 512, (s + 1) * 512)
            for dh in range(2):
                nc.tensor.matmul(
                    q_ps[:, sl],
                    w1t_sb[:, dh, :],
                    x1t_sb[:, dh, s * 512 : (s + 1) * 512],
                    start=(dh == 0),
                    stop=(dh == 1),
                )
        # bias add, rounded straight to f32r
        nc.scalar.activation(qt_sb[:], q_ps[:], IDF, bias=b1_sb[:])

    # ---- B: scores -> exp -> (PE-transpose + fused_x1 accum), ss-outer ----
    # Key strips (ss) outer so each 1024-key slice is finished for all row
    # tiles before moving on: its transposed bf16 weights feed the fused_x1
    # accumulation immediately, keeping PE busy while ACT runs the next exps.
    # The transpose of exp(scores) replaces both a scores.T recompute on the
    # PE and a second 8.4M-element exp pass on ACT.
    with (
        tc.tile_pool(name="psB", bufs=2, space="PSUM") as psB,
        tc.tile_pool(name="psR", bufs=2, space="PSUM") as psR,
        tc.tile_pool(name="psU", bufs=1, space="PSUM") as psU,
        tc.tile_pool(name="exts", bufs=1) as exts,
    ):
        u1t_ps = psU.tile([128, R], F32)
        for ss in range(8):
            exT = exts.tile([128, 8, R], BF16, tag="exT", bufs=2, name="exT")

            def emit_tr_copy(rt):
                # transpose exp row tile rt of this strip into exT columns
                trP = psR.tile([128, 8, 128], BF16, tag="trP", name="trP")
                for k in range(8):
                    nc.tensor.transpose(
                        trP[:, k, :],
                        exp_sb[:, rt, ss * 1024 + k * 128 : ss * 1024 + (k + 1) * 128],
                        identb[:],
                    )
                nc.vector.tensor_copy(exT[:, :, rt * 128 : (rt + 1) * 128], trP[:])

            def emit_u1t(h):
                # fused_x1 accumulation: bank-aligned 512-row half h
                for k in range(8):
                    kt = ss * 8 + k
                    nc.tensor.matmul(
                        u1t_ps[:, h * 512 : (h + 1) * 512],
                        x2b_sb[:, kt, :],
                        exT[:, k, h * 512 : (h + 1) * 512],
                        start=(kt == 0),
                        stop=(kt == KT - 1),
                    )

            for rt in range(RT):
                pb = psB.tile([128, 1024], F32, tag="pb", name="pb")
                for s in range(2):
                    nc.tensor.matmul(
                        pb[:, s * 512 : (s + 1) * 512],
                        qt_sb[:, rt * 128 : (rt + 1) * 128],
                        x2t_sb[:, ss * 1024 + s * 512 : ss * 1024 + (s + 1) * 512],
                        start=True,
                        stop=True,
                    )
                nc.scalar.activation(
                    exp_sb[:, rt, ss * 1024 : (ss + 1) * 1024],
                    pb[:],
                    EXPF,
                    accum_out=ssum[:, rt, ss : ss + 1],
                )
                # emit the previous row tile's transpose+copy so it never
                # head-blocks this tile's matmuls in the in-order PE stream
                if rt > 0:
                    emit_tr_copy(rt - 1)
                if rt == 5:
                    emit_u1t(0)
                if ss == 7:
                    nc.vector.tensor_reduce(
                        sums[:, rt : rt + 1],
                        ssum[:, rt, :],
                        axis=mybir.AxisListType.X,
                        op=mybir.AluOpType.add,
                    )
                    nc.vector.reciprocal(inv[:, rt : rt + 1], sums[:, rt : rt + 1])
                    nc.vector.tensor_scalar_mul(
                        x1b_sb[:, rt, :],
                        x1b_sb[:, rt, :],
                        inv[:, rt : rt + 1],
                    )
            emit_tr_copy(RT - 1)
            emit_u1t(1)
        nc.scalar.copy(u1sb[:], u1t_ps[:])

    # ---- D: fused_x2.T partials; fused_x1 finish; ReduceScatters ----
    with tc.tile_pool(name="fin", bufs=2) as fin:

        def emit_rs(kh):
            if not no_rs:
                nc.gpsimd.collective_compute(
                    "ReduceScatter",
                    mybir.AluOpType.add,
                    replica_groups=[list(range(NC))],
                    ins=[fx2t_d[kh].opt()],
                    outs=[rs_out[kh].opt()],
                )
                nc.sync.dma_start(
                    fx2s_io[:, kh * (N2 // 4) : (kh + 1) * (N2 // 4)],
                    rs_out[kh][:],
                )
            else:
                nc.sync.dma_start(
                    fx2s_io[:, kh * (N2 // 4) : (kh + 1) * (N2 // 4)],
                    fx2t_d[kh][0 : D1 // NC, :],
                )

        with tc.tile_pool(name="psD", bufs=2, space="PSUM") as psD:
            for g in range(16):
                q, dh, s2 = g // 4, (g // 2) % 2, g % 2
                koff = q * 2048 + s2 * 1024
                pd = psD.tile([128, 1024], F32, tag="pd", name="pd")
                for rt in range(RT):
                    for s in range(2):
                        nc.tensor.matmul(
                            pd[:, s * 512 : (s + 1) * 512],
                            x1b_sb[:, rt, dh * 128 : (dh + 1) * 128],
                            exp_sb[:, rt, koff + s * 512 : koff + (s + 1) * 512],
                            start=(rt == 0),
                            stop=(rt == RT - 1),
                        )
                fx2st = fin.tile([128, 1024], F32, tag="fx2st", bufs=2, name="fx2st")
                if g % 2 == 0:
                    nc.vector.tensor_copy(fx2st[:], pd[:])
                else:
                    nc.scalar.copy(fx2st[:], pd[:])
                nc.sync.dma_start(
                    fx2t_d[q][dh * 128 : (dh + 1) * 128, s2 * 1024 : (s2 + 1) * 1024],
                    fx2st[:],
                )
                if g % 4 == 3 and g < 15:
                    emit_rs(g // 4)

        # fused_x1: transpose the unnormalized U1T row tiles and scale
        with tc.tile_pool(name="psF", bufs=2, space="PSUM") as psF:
            for rt in range(RT):
                ptr = psF.tile([128, 128], F32, tag="ptr", name="ptr")
                nc.tensor.transpose(ptr[:], u1sb[:, rt * 128 : (rt + 1) * 128], ident[:])
                fx1st = fin.tile([128, 128], F32, tag="fx1st", name="fx1st")
                nc.vector.tensor_scalar_mul(fx1st[:], ptr[:], inv[:, rt : rt + 1])
                nc.sync.dma_start(fx1_io[rt * 128 : (rt + 1) * 128, :], fx1st[:])

        emit_rs(3)


def _get_nc(rep=1, no_rs=False):
    key = (rep, no_rs)
    if key not in _CACHED:
        _CACHED[key] = _build(rep, no_rs=no_rs)
    return _CACHED[key]


def _prep_in_maps(x1, x2, W1, b1):
    x1 = np.asarray(x1, dtype=np.float32)
    x2 = np.asarray(x2, dtype=np.float32)
    W1 = np.asarray(W1, dtype=np.float32)
    b1 = np.asarray(b1, dtype=np.float32)

    x2t = np.ascontiguousarray(x2.T.astype(np.float16))
    x2b = np.ascontiguousarray(x2.astype(ml_dtypes.bfloat16))
    w1t = np.ascontiguousarray(W1.T)
    b1c = np.ascontiguousarray(b1.reshape(D2, 1))

    in_maps = []
    for i in range(NC):
        sh = x1[i * R : (i + 1) * R]
        in_maps.append(
            {
                "x1t": np.ascontiguousarray(sh.T),
                "x1b": np.ascontiguousarray(sh.astype(ml_dtypes.bfloat16)),
                "x2t": x2t,
                "x2b": x2b,
                "w1t": w1t,
                "b1": b1c,
            }
        )
    return in_maps


def _assemble(res):
    fused_x1 = np.concatenate([r["fx1"] for r in res], axis=0)
    fused_x2 = np.ascontiguousarray(
        np.concatenate([r["fx2s"] for r in res], axis=0).T
    )
    return fused_x1.astype(np.float32), fused_x2.astype(np.float32)


def kernel(x1, x2, W1, b1):
    nc = _get_nc()
    in_maps = _prep_in_maps(x1, x2, W1, b1)
    res = run_bass_kernel_spmd(nc, in_maps, list(range(NC))).results
    return _assemble(res)


# revision 18
# speedup vs baseline: 1.1691x; 1.1691x over previous
"""Trainium2 Bass kernel for nn_CrossAttentionLayer (8-core SPMD).

  q  = x1 @ W1.T + b1            [N1, D2]
  attn = softmax(q @ x2.T, dim=1)
  fused_x1 = attn @ x2           [N1, D2]
  fused_x2 = attn.T @ x1         [N2, D1]   (ReduceScatter over row shards)

Sharding: x1 rows split across 8 cores (1024 rows each); x2/W1 replicated.
Per core: scores computed twice on the PE (row-major for fused_x2's operand,
transposed per key-tile for fused_x1) in float32r; exp on ACT; attention
weights kept unnormalized in bf16, with softmax row sums obtained for free
from the ACT accum_out, applied as inv-sum scaling of x1 (fused_x2) and of
the fused_x1 result after an on-PE transpose. fused_x2 partials are
ReduceScattered on-device; the host concatenates the per-core shards.
"""

import sys

if "/opt/trn_rl_repo" not in sys.path:
    sys.path.insert(0, "/opt/trn_rl_repo")

import numpy as np
import ml_dtypes

import concourse.bacc as bacc
import concourse.mybir as mybir
import concourse.tile as tile
from concourse.bass_utils import run_bass_kernel_spmd
from concourse.masks import make_identity

F32 = mybir.dt.float32
F32R = mybir.dt.float32r
BF16 = mybir.dt.bfloat16
FP16 = mybir.dt.float16
EXPF = mybir.ActivationFunctionType.Exp
IDF = mybir.ActivationFunctionType.Identity

N1, N2 = 8192, 8192
D1, D2 = 256, 128
NC = 8
R = N1 // NC          # rows per core: 1024
RT = R // 128         # row tiles per core: 8
KT = N2 // 128        # key tiles: 64
KS = N2 // 512        # 512-wide key strips: 16

_CACHED = {}


def _build(rep=1, for_sim=False, no_rs=False):
    no_rs = no_rs or for_sim
    nd = 1 if for_sim else NC
    nc = bacc.Bacc("TRN2", target_bir_lowering=False, debug=False, num_devices=nd)

    # Kernel I/O (per core). f32r tensors receive raw fp32 bytes; the PE's
    # f32r mode reads them at full rate (validated: same accuracy as a DVE
    # round-to-f32r pass).
    x1t_io = nc.dram_tensor("x1t", [D1, R], F32R, kind="ExternalInput").ap()
    x1b_io = nc.dram_tensor("x1b", [R, D1], BF16, kind="ExternalInput").ap()
    x2t_io = nc.dram_tensor("x2t", [D2, N2], FP16, kind="ExternalInput").ap()
    x2b_io = nc.dram_tensor("x2b", [N2, D2], BF16, kind="ExternalInput").ap()
    w1t_io = nc.dram_tensor("w1t", [D1, D2], F32R, kind="ExternalInput").ap()
    b1_io = nc.dram_tensor("b1", [D2, 1], F32, kind="ExternalInput").ap()
    fx1_io = nc.dram_tensor("fx1", [R, D2], F32, kind="ExternalOutput").ap()
    fx2s_io = nc.dram_tensor("fx2s", [D1 // NC, N2], F32, kind="ExternalOutput").ap()

    with tile.TileContext(nc) as tc:
        with (
            tc.tile_pool(name="pers", bufs=1) as pers,
            tc.tile_pool(name="dram", bufs=1, space="DRAM") as drp,
        ):
            # ---- persistent SBUF ----
            x2t_sb = pers.tile([128, N2], FP16)           # x2.T  (j part, keys free)
            x2b_sb = pers.tile([128, KT, D2], BF16)       # x2    (key%128 part, kt, j)
            x1b_sb = pers.tile([128, RT, D1], BF16)       # x1, scaled by inv_sum in place
            qt_sb = pers.tile([128, R], FP16)             # q.T   (j part, rows free)
            exp_sb = pers.tile([128, RT, N2], BF16)       # exp(scores) (row%128, rt, key)
            ssum = pers.tile([128, RT, 8], F32)           # per-strip exp sums
            sums = pers.tile([128, RT], F32)
            inv = pers.tile([128, RT], F32)
            ident = pers.tile([128, 128], F32)
            identb = pers.tile([128, 128], BF16)
            u1sb = pers.tile([128, R], F32)               # unnormalized fused_x1.T

            # fused_x2.T partials & ReduceScatter buffers (split in key
            # quarters so collectives overlap the tail of D)
            fx2t_d = [drp.tile([D1, N2 // 4], F32, name=f"fx2t_{h}") for h in range(4)]
            rs_out = [drp.tile([D1 // NC, N2 // 4], F32, name=f"rso_{h}") for h in range(4)]

            make_identity(nc, ident[:])
            make_identity(nc, identb[:])

            for ri in range(rep):
                _emit_iter(
                    nc, tc, pers,
                    x1t_io, x1b_io, x2t_io, x2b_io, w1t_io, b1_io,
                    fx1_io, fx2s_io,
                    x2t_sb, x2b_sb, x1b_sb, qt_sb, exp_sb,
                    ssum, sums, inv, ident, identb, u1sb, fx2t_d, rs_out,
                    no_rs,
                )

    nc.compile()
    return nc


def _emit_iter(
    nc, tc, pers,
    x1t_io, x1b_io, x2t_io, x2b_io, w1t_io, b1_io, fx1_io, fx2s_io,
    x2t_sb, x2b_sb, x1b_sb, qt_sb, exp_sb,
    ssum, sums, inv, ident, identb, u1sb, fx2t_d, rs_out,
    no_rs=False,
):
    with (
        tc.tile_pool(name="stage", bufs=1) as stage,
        tc.tile_pool(name="psQ", bufs=1, space="PSUM") as psQ,
    ):
        x1t_sb = stage.tile([128, 2, R], F32R)
        w1t_sb = stage.tile([128, 2, D2], F32R)
        b1_sb = stage.tile([128, 1], F32)
        # q-critical loads first, then x2t in chunks so the first score
        # matmuls start early; bulk loads for later phases go via gpsimd.
        nc.sync.dma_start(w1t_sb[:], w1t_io.rearrange("(dh p) j -> p dh j", p=128))
        nc.sync.dma_start(b1_sb[:], b1_io)
        nc.sync.dma_start(x1t_sb[:], x1t_io.rearrange("(dh p) m -> p dh m", p=128))
        for c in range(8):
            nc.sync.dma_start(
                x2t_sb[:, c * 1024 : (c + 1) * 1024],
                x2t_io[:, c * 1024 : (c + 1) * 1024],
            )
        nc.gpsimd.dma_start(x1b_sb[:], x1b_io.rearrange("(rt p) d -> p rt d", p=128))
        nc.gpsimd.dma_start(x2b_sb[:], x2b_io.rearrange("(kt p) j -> p kt j", p=128))

        # ---- q.T = W1T.T @ x1T + b1 : [128 j, 1024 m] ----
        q_ps = psQ.tile([128, R], F32)
        for s in range(2):
            sl = slice(s * 512, (s + 1) * 512)
            for dh in range(2):
                nc.tensor.matmul(
                    q_ps[:, sl],
                    w1t_sb[:, dh, :],
                    x1t_sb[:, dh, s * 512 : (s + 1) * 512],
                    start=(dh == 0),
                    stop=(dh == 1),
                )
        # bias add, rounded straight to f32r
        nc.scalar.activation(qt_sb[:], q_ps[:], IDF, bias=b1_sb[:])

    # ---- B: scores -> exp -> (PE-transpose + fused_x1 accum), ss-outer ----
    # Key strips (ss) outer so each 1024-key slice is finished for all row
    # tiles before moving on: its transposed bf16 weights feed the fused_x1
    # accumulation immediately, keeping PE busy while ACT runs the next exps.
    # The transpose of exp(scores) replaces both a scores.T recompute on the
    # PE and a second 8.4M-element exp pass on ACT.
    with (
        tc.tile_pool(name="psB", bufs=2, space="PSUM") as psB,
        tc.tile_pool(name="psR", bufs=2, space="PSUM") as psR,
        tc.tile_pool(name="psU", bufs=1, space="PSUM") as psU,
        tc.tile_pool(name="exts", bufs=1) as exts,
    ):
        u1t_ps = psU.tile([128, R], F32)
        pending_u1t = []

        def drain_u1t(n):
            for _ in range(min(n, len(pending_u1t))):
                pending_u1t.pop(0)()

        for ss in range(8):
            exT = exts.tile([128, 8, R], BF16, tag="exT", bufs=2, name="exT")

            def emit_tr_copy(rt, ss=ss, exT=exT):
                trP = psR.tile([128, 8, 128], BF16, tag="trP", name="trP")
                for k in range(8):
                    nc.tensor.transpose(
                        trP[:, k, :],
                        exp_sb[:, rt, ss * 1024 + k * 128 : ss * 1024 + (k + 1) * 128],
                        identb[:],
                    )
                nc.vector.tensor_copy(exT[:, :, rt * 128 : (rt + 1) * 128], trP[:])

            def queue_u1t(h, ss=ss, exT=exT):
                # fused_x1 accumulation MMs for bank-aligned row half h of
                # this strip; drained two per row tile so they never
                # head-block the score matmuls in the in-order PE stream.
                for k in range(8):
                    kt = ss * 8 + k

                    def mm(kt=kt, k=k, h=h, exT=exT):
                        nc.tensor.matmul(
                            u1t_ps[:, h * 512 : (h + 1) * 512],
                            x2b_sb[:, kt, :],
                            exT[:, k, h * 512 : (h + 1) * 512],
                            start=(kt == 0),
                            stop=(kt == KT - 1),
                        )

                    pending_u1t.append(mm)

            for rt in range(RT):
                pb = psB.tile([128, 1024], F32, tag="pb", name="pb")
                for s in range(2):
                    nc.tensor.matmul(
                        pb[:, s * 512 : (s + 1) * 512],
                        qt_sb[:, rt * 128 : (rt + 1) * 128],
                        x2t_sb[:, ss * 1024 + s * 512 : ss * 1024 + (s + 1) * 512],
                        start=True,
                        stop=True,
                    )
                nc.scalar.activation(
                    exp_sb[:, rt, ss * 1024 : (ss + 1) * 1024],
                    pb[:],
                    EXPF,
                    accum_out=ssum[:, rt, ss : ss + 1],
                )
                if rt > 0:
                    emit_tr_copy(rt - 1)
                    if rt == 5:
                        queue_u1t(0)
                drain_u1t(2)
                if ss == 7:
                    nc.vector.tensor_reduce(
                        sums[:, rt : rt + 1],
                        ssum[:, rt, :],
                        axis=mybir.AxisListType.X,
                        op=mybir.AluOpType.add,
                    )
                    nc.vector.reciprocal(inv[:, rt : rt + 1], sums[:, rt : rt + 1])
                    nc.vector.tensor_scalar_mul(
                        x1b_sb[:, rt, :],
                        x1b_sb[:, rt, :],
                        inv[:, rt : rt + 1],
                    )
            emit_tr_copy(RT - 1)
            queue_u1t(1)
        drain_u1t(len(pending_u1t))
        nc.scalar.copy(u1sb[:], u1t_ps[:])

    # ---- D: fused_x2.T partials; fused_x1 finish; ReduceScatters ----
    with tc.tile_pool(name="fin", bufs=2) as fin:

        def emit_rs(kh):
            if not no_rs:
                nc.gpsimd.collective_compute(
                    "ReduceScatter",
                    mybir.AluOpType.add,
                    replica_groups=[list(range(NC))],
                    ins=[fx2t_d[kh].opt()],
                    outs=[rs_out[kh].opt()],
                )
                nc.sync.dma_start(
                    fx2s_io[:, kh * (N2 // 4) : (kh + 1) * (N2 // 4)],
                    rs_out[kh][:],
                )
            else:
                nc.sync.dma_start(
                    fx2s_io[:, kh * (N2 // 4) : (kh + 1) * (N2 // 4)],
                    fx2t_d[kh][0 : D1 // NC, :],
                )

        with tc.tile_pool(name="psD", bufs=2, space="PSUM") as psD:
            for g in range(16):
                q, dh, s2 = g // 4, (g // 2) % 2, g % 2
                koff = q * 2048 + s2 * 1024
                pd = psD.tile([128, 1024], F32, tag="pd", name="pd")
                for rt in range(RT):
                    for s in range(2):
                        nc.tensor.matmul(
                            pd[:, s * 512 : (s + 1) * 512],
                            x1b_sb[:, rt, dh * 128 : (dh + 1) * 128],
                            exp_sb[:, rt, koff + s * 512 : koff + (s + 1) * 512],
                            start=(rt == 0),
                            stop=(rt == RT - 1),
                        )
                fx2st = fin.tile([128, 1024], F32, tag="fx2st", bufs=2, name="fx2st")
                if g % 2 == 0:
                    nc.vector.tensor_copy(fx2st[:], pd[:])
                else:
                    nc.scalar.copy(fx2st[:], pd[:])
                nc.sync.dma_start(
                    fx2t_d[q][dh * 128 : (dh + 1) * 128, s2 * 1024 : (s2 + 1) * 1024],
                    fx2st[:],
                )
                if g % 4 == 3 and g < 15:
                    emit_rs(g // 4)

        # fused_x1: transpose the unnormalized U1T row tiles and scale
        with tc.tile_pool(name="psF", bufs=2, space="PSUM") as psF:
            for rt in range(RT):
                ptr = psF.tile([128, 128], F32, tag="ptr", name="ptr")
                nc.tensor.transpose(ptr[:], u1sb[:, rt * 128 : (rt + 1) * 128], ident[:])
                fx1st = fin.tile([128, 128], F32, tag="fx1st", name="fx1st")
                nc.vector.tensor_scalar_mul(fx1st[:], ptr[:], inv[:, rt : rt + 1])
                nc.sync.dma_start(fx1_io[rt * 128 : (rt + 1) * 128, :], fx1st[:])

        emit_rs(3)


def _get_nc(rep=1, no_rs=False):
    key = (rep, no_rs)
    if key not in _CACHED:
        _CACHED[key] = _build(rep, no_rs=no_rs)
    return _CACHED[key]


def _prep_in_maps(x1, x2, W1, b1):
    x1 = np.asarray(x1, dtype=np.float32)
    x2 = np.asarray(x2, dtype=np.float32)
    W1 = np.asarray(W1, dtype=np.float32)
    b1 = np.asarray(b1, dtype=np.float32)

    x2t = np.ascontiguousarray(x2.T.astype(np.float16))
    x2b = np.ascontiguousarray(x2.astype(ml_dtypes.bfloat16))
    w1t = np.ascontiguousarray(W1.T)
    b1c = np.ascontiguousarray(b1.reshape(D2, 1))

    in_maps = []
    for i in range(NC):
        sh = x1[i * R : (i + 1) * R]
        in_maps.append(
            {
                "x1t": np.ascontiguousarray(sh.T),
                "x1b": np.ascontiguousarray(sh.astype(ml_dtypes.bfloat16)),
                "x2t": x2t,
                "x2b": x2b,
                "w1t": w1t,
                "b1": b1c,
            }
        )
    return in_maps


def _assemble(res):
    fused_x1 = np.concatenate([r["fx1"] for r in res], axis=0)
    fused_x2 = np.ascontiguousarray(
        np.concatenate([r["fx2s"] for r in res], axis=0).T
    )
    return fused_x1.astype(np.float32), fused_x2.astype(np.float32)


def kernel(x1, x2, W1, b1):
    nc = _get_nc()
    in_maps = _prep_in_maps(x1, x2, W1, b1)
    res = run_bass_kernel_spmd(nc, in_maps, list(range(NC))).results
    return _assemble(res)


# revision 19
# speedup vs baseline: 2.2293x; 1.9068x over previous
"""Trainium2 Bass kernel for nn_CrossAttentionLayer (8-core SPMD).

  q  = x1 @ W1.T + b1            [N1, D2]
  attn = softmax(q @ x2.T, dim=1)
  fused_x1 = attn @ x2           [N1, D2]
  fused_x2 = attn.T @ x1         [N2, D1]   (ReduceScatter over row shards)

Sharding: x1 rows split across 8 cores (1024 rows each); x2/W1 replicated.
Per core: scores computed twice on the PE (row-major for fused_x2's operand,
transposed per key-tile for fused_x1) in float32r; exp on ACT; attention
weights kept unnormalized in bf16, with softmax row sums obtained for free
from the ACT accum_out, applied as inv-sum scaling of x1 (fused_x2) and of
the fused_x1 result after an on-PE transpose. fused_x2 partials are
ReduceScattered on-device; the host concatenates the per-core shards.
"""

import sys

if "/opt/trn_rl_repo" not in sys.path:
    sys.path.insert(0, "/opt/trn_rl_repo")

import numpy as np
import ml_dtypes

import concourse.bacc as bacc
import concourse.mybir as mybir
import concourse.tile as tile
from concourse.bass_utils import run_bass_kernel_spmd
from concourse.masks import make_identity

F32 = mybir.dt.float32
F32R = mybir.dt.float32r
BF16 = mybir.dt.bfloat16
FP16 = mybir.dt.float16
EXPF = mybir.ActivationFunctionType.Exp
IDF = mybir.ActivationFunctionType.Identity

N1, N2 = 8192, 8192
D1, D2 = 256, 128
NC = 8
R = N1 // NC          # rows per core: 1024
RT = R // 128         # row tiles per core: 8
KT = N2 // 128        # key tiles: 64
KS = N2 // 512        # 512-wide key strips: 16

_CACHED = {}


def _build(rep=1, for_sim=False, no_rs=False):
    no_rs = no_rs or for_sim
    nd = 1 if for_sim else NC
    nc = bacc.Bacc("TRN2", target_bir_lowering=False, debug=False, num_devices=nd)

    # Kernel I/O (per core). f32r tensors receive raw fp32 bytes; the PE's
    # f32r mode reads them at full rate (validated: same accuracy as a DVE
    # round-to-f32r pass).
    x1t_io = nc.dram_tensor("x1t", [D1, R], F32R, kind="ExternalInput").ap()
    x1b_io = nc.dram_tensor("x1b", [R, D1], BF16, kind="ExternalInput").ap()
    x2t_io = nc.dram_tensor("x2t", [D2, N2], FP16, kind="ExternalInput").ap()
    x2b_io = nc.dram_tensor("x2b", [N2, D2], BF16, kind="ExternalInput").ap()
    w1t_io = nc.dram_tensor("w1t", [D1, D2], F32R, kind="ExternalInput").ap()
    b1_io = nc.dram_tensor("b1", [D2, 1], F32, kind="ExternalInput").ap()
    fx1_io = nc.dram_tensor("fx1", [R, D2], F32, kind="ExternalOutput").ap()
    fx2s_io = nc.dram_tensor("fx2s", [D1 // NC, N2], F32, kind="ExternalOutput").ap()

    with tile.TileContext(nc) as tc:
        with (
            tc.tile_pool(name="pers", bufs=1) as pers,
            tc.tile_pool(name="dram", bufs=1, space="DRAM") as drp,
        ):
            # ---- persistent SBUF ----
            x2t_sb = pers.tile([128, N2], FP16)           # x2.T  (j part, keys free)
            x2b_sb = pers.tile([128, KT, D2], BF16)       # x2    (key%128 part, kt, j)
            x1b_sb = pers.tile([128, RT, D1], BF16)       # x1, scaled by inv_sum in place
            qt_sb = pers.tile([128, R], FP16)             # q.T   (j part, rows free)
            exp_sb = pers.tile([128, RT, N2], BF16)       # exp(scores) (row%128, rt, key)
            ssum = pers.tile([128, RT, 8], F32)           # per-strip exp sums
            sums = pers.tile([128, RT], F32)
            inv = pers.tile([128, RT], F32)
            ident = pers.tile([128, 128], F32)
            identb = pers.tile([128, 128], BF16)
            u1sb = pers.tile([128, R], F32)               # unnormalized fused_x1.T

            # fused_x2.T partials & ReduceScatter buffers (split in key
            # quarters so collectives overlap the tail of D)
            fx2t_d = [drp.tile([D1, N2 // 4], F32, name=f"fx2t_{h}") for h in range(4)]
            rs_out = [drp.tile([D1 // NC, N2 // 4], F32, name=f"rso_{h}") for h in range(4)]

            make_identity(nc, ident[:])
            make_identity(nc, identb[:])

            for ri in range(rep):
                _emit_iter(
                    nc, tc, pers,
                    x1t_io, x1b_io, x2t_io, x2b_io, w1t_io, b1_io,
                    fx1_io, fx2s_io,
                    x2t_sb, x2b_sb, x1b_sb, qt_sb, exp_sb,
                    ssum, sums, inv, ident, identb, u1sb, fx2t_d, rs_out,
                    no_rs,
                )

    nc.compile()
    return nc


def _emit_iter(
    nc, tc, pers,
    x1t_io, x1b_io, x2t_io, x2b_io, w1t_io, b1_io, fx1_io, fx2s_io,
    x2t_sb, x2b_sb, x1b_sb, qt_sb, exp_sb,
    ssum, sums, inv, ident, identb, u1sb, fx2t_d, rs_out,
    no_rs=False,
):
    with (
        tc.tile_pool(name="stage", bufs=1) as stage,
        tc.tile_pool(name="psQ", bufs=1, space="PSUM") as psQ,
    ):
        x1t_sb = stage.tile([128, 2, R], F32R)
        w1t_sb = stage.tile([128, 2, D2], F32R)
        b1_sb = stage.tile([128, 1], F32)
        # q-critical loads first, then x2t in chunks so the first score
        # matmuls start early; bulk loads for later phases go via gpsimd.
        nc.sync.dma_start(w1t_sb[:], w1t_io.rearrange("(dh p) j -> p dh j", p=128))
        nc.sync.dma_start(b1_sb[:], b1_io)
        nc.sync.dma_start(x1t_sb[:], x1t_io.rearrange("(dh p) m -> p dh m", p=128))
        for c in range(8):
            nc.sync.dma_start(
                x2t_sb[:, c * 1024 : (c + 1) * 1024],
                x2t_io[:, c * 1024 : (c + 1) * 1024],
            )
        nc.gpsimd.dma_start(x2b_sb[:], x2b_io.rearrange("(kt p) j -> p kt j", p=128))
        nc.gpsimd.dma_start(x1b_sb[:], x1b_io.rearrange("(rt p) d -> p rt d", p=128))

        # ---- q.T = W1T.T @ x1T + b1 : [128 j, 1024 m] ----
        q_ps = psQ.tile([128, R], F32)
        for s in range(2):
            sl = slice(s * 512, (s + 1) * 512)
            for dh in range(2):
                nc.tensor.matmul(
                    q_ps[:, sl],
                    w1t_sb[:, dh, :],
                    x1t_sb[:, dh, s * 512 : (s + 1) * 512],
                    start=(dh == 0),
                    stop=(dh == 1),
                )
        # bias add, rounded straight to f32r
        nc.scalar.activation(qt_sb[:], q_ps[:], IDF, bias=b1_sb[:])

    # ---- B: scores -> exp -> (PE-transpose + fused_x1 accum), ss-outer ----
    # Key strips (ss) outer so each 1024-key slice is finished for all row
    # tiles before moving on: its transposed bf16 weights feed the fused_x1
    # accumulation immediately, keeping PE busy while ACT runs the next exps.
    # The transpose of exp(scores) replaces both a scores.T recompute on the
    # PE and a second 8.4M-element exp pass on ACT.
    with (
        tc.tile_pool(name="psB", bufs=2, space="PSUM") as psB,
        tc.tile_pool(name="psR", bufs=2, space="PSUM") as psR,
        tc.tile_pool(name="psU", bufs=1, space="PSUM") as psU,
        tc.tile_pool(name="exts", bufs=1) as exts,
    ):
        u1t_ps = psU.tile([128, R], F32)
        pending_u1t = []

        def drain_u1t(n):
            for _ in range(min(n, len(pending_u1t))):
                pending_u1t.pop(0)()

        for ss in range(8):
            exT = exts.tile([128, 8, R], BF16, tag="exT", bufs=2, name="exT")

            def emit_tr_copy(rt, ss=ss, exT=exT):
                trP = psR.tile([128, 8, 128], BF16, tag="trP", name="trP")
                for k in range(8):
                    nc.tensor.transpose(
                        trP[:, k, :],
                        exp_sb[:, rt, ss * 1024 + k * 128 : ss * 1024 + (k + 1) * 128],
                        identb[:],
                    )
                nc.vector.tensor_copy(exT[:, :, rt * 128 : (rt + 1) * 128], trP[:])

            def queue_u1t(h, ss=ss, exT=exT):
                # fused_x1 accumulation MMs for bank-aligned row half h of
                # this strip; drained two per row tile so they never
                # head-block the score matmuls in the in-order PE stream.
                for k in range(8):
                    kt = ss * 8 + k

                    def mm(kt=kt, k=k, h=h, exT=exT):
                        nc.tensor.matmul(
                            u1t_ps[:, h * 512 : (h + 1) * 512],
                            x2b_sb[:, kt, :],
                            exT[:, k, h * 512 : (h + 1) * 512],
                            start=(kt == 0),
                            stop=(kt == KT - 1),
                        )

                    pending_u1t.append(mm)

            for rt in range(RT):
                pb = psB.tile([128, 1024], F32, tag="pb", name="pb")
                for s in range(2):
                    nc.tensor.matmul(
                        pb[:, s * 512 : (s + 1) * 512],
                        qt_sb[:, rt * 128 : (rt + 1) * 128],
                        x2t_sb[:, ss * 1024 + s * 512 : ss * 1024 + (s + 1) * 512],
                        start=True,
                        stop=True,
                    )
                nc.scalar.activation(
                    exp_sb[:, rt, ss * 1024 : (ss + 1) * 1024],
                    pb[:],
                    EXPF,
                    accum_out=ssum[:, rt, ss : ss + 1],
                )
                if rt > 0:
                    emit_tr_copy(rt - 1)
                    if rt == 5:
                        queue_u1t(0)
                drain_u1t(2)
                if ss == 7:
                    nc.vector.tensor_reduce(
                        sums[:, rt : rt + 1],
                        ssum[:, rt, :],
                        axis=mybir.AxisListType.X,
                        op=mybir.AluOpType.add,
                    )
                    nc.vector.reciprocal(inv[:, rt : rt + 1], sums[:, rt : rt + 1])
                    nc.vector.tensor_scalar_mul(
                        x1b_sb[:, rt, :],
                        x1b_sb[:, rt, :],
                        inv[:, rt : rt + 1],
                    )
            emit_tr_copy(RT - 1)
            queue_u1t(1)
        drain_u1t(len(pending_u1t))
        nc.scalar.copy(u1sb[:], u1t_ps[:])

    # ---- D: fused_x2.T partials; fused_x1 finish; ReduceScatters ----
    with tc.tile_pool(name="fin", bufs=2) as fin:

        def emit_rs(kh):
            if not no_rs:
                nc.gpsimd.collective_compute(
                    "ReduceScatter",
                    mybir.AluOpType.add,
                    replica_groups=[list(range(NC))],
                    ins=[fx2t_d[kh].opt()],
                    outs=[rs_out[kh].opt()],
                )
                nc.sync.dma_start(
                    fx2s_io[:, kh * (N2 // 4) : (kh + 1) * (N2 // 4)],
                    rs_out[kh][:],
                )
            else:
                nc.sync.dma_start(
                    fx2s_io[:, kh * (N2 // 4) : (kh + 1) * (N2 // 4)],
                    fx2t_d[kh][0 : D1 // NC, :],
                )

        # fused_x1: transpose the unnormalized U1T row tiles and scale
        with tc.tile_pool(name="psF", bufs=2, space="PSUM") as psF:
            for rt in range(RT):
                ptr = psF.tile([128, 128], F32, tag="ptr", name="ptr")
                nc.tensor.transpose(ptr[:], u1sb[:, rt * 128 : (rt + 1) * 128], ident[:])
                fx1st = fin.tile([128, 128], F32, tag="fx1st", name="fx1st")
                nc.vector.tensor_scalar_mul(fx1st[:], ptr[:], inv[:, rt : rt + 1])
                nc.sync.dma_start(fx1_io[rt * 128 : (rt + 1) * 128, :], fx1st[:])

        with tc.tile_pool(name="psD", bufs=2, space="PSUM") as psD:
            for g in range(16):
                q, dh, s2 = g // 4, (g // 2) % 2, g % 2
                koff = q * 2048 + s2 * 1024
                pd = psD.tile([128, 1024], F32, tag="pd", name="pd")
                for rt in range(RT):
                    for s in range(2):
                        nc.tensor.matmul(
                            pd[:, s * 512 : (s + 1) * 512],
                            x1b_sb[:, rt, dh * 128 : (dh + 1) * 128],
                            exp_sb[:, rt, koff + s * 512 : koff + (s + 1) * 512],
                            start=(rt == 0),
                            stop=(rt == RT - 1),
                        )
                fx2st = fin.tile([128, 1024], F32, tag="fx2st", bufs=2, name="fx2st")
                if g % 2 == 0:
                    nc.vector.tensor_copy(fx2st[:], pd[:])
                else:
                    nc.scalar.copy(fx2st[:], pd[:])
                nc.sync.dma_start(
                    fx2t_d[q][dh * 128 : (dh + 1) * 128, s2 * 1024 : (s2 + 1) * 1024],
                    fx2st[:],
                )
                if g % 4 == 3 and g < 15:
                    emit_rs(g // 4)

        emit_rs(3)


def _get_nc(rep=1, no_rs=False):
    key = (rep, no_rs)
    if key not in _CACHED:
        _CACHED[key] = _build(rep, no_rs=no_rs)
    return _CACHED[key]


def _prep_in_maps(x1, x2, W1, b1):
    x1 = np.asarray(x1, dtype=np.float32)
    x2 = np.asarray(x2, dtype=np.float32)
    W1 = np.asarray(W1, dtype=np.float32)
    b1 = np.asarray(b1, dtype=np.float32)

    x2t = np.ascontiguousarray(x2.T.astype(np.float16))
    x2b = np.ascontiguousarray(x2.astype(ml_dtypes.bfloat16))
    w1t = np.ascontiguousarray(W1.T)
    b1c = np.ascontiguousarray(b1.reshape(D2, 1))

    in_maps = []
    for i in range(NC):
        sh = x1[i * R : (i + 1) * R]
        in_maps.append(
            {
                "x1t": np.ascontiguousarray(sh.T),
                "x1b": np.ascontiguousarray(sh.astype(ml_dtypes.bfloat16)),
                "x2t": x2t,
                "x2b": x2b,
                "w1t": w1t,
                "b1": b1c,
            }
        )
    return in_maps


def _assemble(res):
    fused_x1 = np.concatenate([r["fx1"] for r in res], axis=0)
    fused_x2 = np.ascontiguousarray(
        np.concatenate([r["fx2s"] for r in res], axis=0).T
    )
    return fused_x1.astype(np.float32), fused_x2.astype(np.float32)


def kernel(x1, x2, W1, b1):
    nc = _get_nc()
    in_maps = _prep_in_maps(x1, x2, W1, b1)
    res = run_bass_kernel_spmd(nc, in_maps, list(range(NC))).results
    return _assemble(res)
